# revision 11
# baseline (speedup 1.0000x reference)
"""Causal single-head attention (B=8, T=2048, D=128, H=16) on 8 Trainium2 cores.

Strategy (v3): data-parallel over batch (1 element per core). Per core:
  - Host precomputes M = Wq @ Wk^T [128,128]; the device computes
    u = M^T x (one matmul per x chunk), then score tiles
    ST[k, q] = xT_ktile^T @ u with K=128 contraction: no separate q/k
    projections, and x tiles serve directly as the ST stationary.
  - exp of each score group runs entirely on ONE of ACT/DVE, chosen by a
    greedy load balancer (whole-group: two engines writing one pt tile
    would serialize on a tile-granular WAW dependency). ACT groups use
    exact exp with a +ln(1.0407) bias so their systematic mean matches the
    DVE groups' piecewise-linear exp (i16 = round(score*scale*log2e*1024 +
    15360) bitcast f16); the common mean cancels in the softmax division.
    PSUM->SBUF copies (u/v/o) are balanced across ACT/DVE the same way.
  - Causal masking: odd-diagonal key tiles computed at half width; the two
    [128,128] triangular masks per query block applied in-place on the f16
    probability tiles by GPSIMD affine_select (idle engine).
  - PV uses pt as the matmul *stationary* and v-tiles [128, 17] (ones
    column -> softmax denominator) as 17-column moving operands. Output
    accumulates in one PSUM bank as 16 [128,17] regions; all matmuls of
    one region are emitted contiguously (PSUM allows only one open
    accumulation group per bank). Natural [T, 17] output layout, batched
    DMAs issued from the same engine queue as the preceding copy.
  - Host divides by the denominator column. rel l2 err ~3e-3.
"""

import os

import numpy as np

B, T, D, H = 8, 2048, 128, 16
NT = T // 128
SCALE = H ** -0.5
LOG2E = 1.4426950408889634
MULT = SCALE * LOG2E * 1024.0
MAGIC = 15360.0
MEAN_LN = 0.03987866060337333

GROUP = int(os.environ.get("ATT_GROUP", "4"))
PD = int(os.environ.get("ATT_PD", "2"))
ACT_T0 = float(os.environ.get("ATT_ACT_T0", "1583"))
DVE_T0 = float(os.environ.get("ATT_DVE_T0", "0"))

_CACHE = {}


def _groups():
    out = []
    for qb in range(8):
        a, b = 2 * qb, 2 * qb + 1
        tl = [(j, 256, 'f') for j in range(a)] + [(a, 256, 'e'), (b, 128, 'o')]
        ch = [tl[i:i + GROUP] for i in range(0, len(tl), GROUP)]
        ch.reverse()
        if qb == 7 and len(ch[-1]) >= 2:
            last = ch.pop()
            ch.append(last[1:])
            ch.append(last[:1])
        for tiles in ch:
            off = 0
            placed = []
            for (j, wid, kind) in tiles:
                placed.append((j, off, wid, kind))
                off += wid
            out.append(dict(qb=qb, a=a, b=b, tiles=placed, cols=off))
    return out


def _build():
    import concourse.mybir as mybir
    import concourse.tile as tile
    from concourse import bacc

    f32 = mybir.dt.float32
    f16 = mybir.dt.float16
    i16 = mybir.dt.int16
    Exp = mybir.ActivationFunctionType.Exp

    nc = bacc.Bacc()
    xT_d = nc.declare_dram_parameter("xT", [D, T], f16, isOutput=False)
    cst_d = nc.declare_dram_parameter("cst", [128, 144], f16, isOutput=False)
    out_d = nc.declare_dram_parameter("oD", [T, H + 1], f32, isOutput=True)

    groups = _groups()
    n = len(groups)
    GW = GROUP * 256

    # x / u chunk layout (finer early chunks fill the pipeline sooner)
    CH = [(0, 256), (256, 256), (512, 256), (768, 512), (1280, 512),
          (1792, 256)]
    u_at = {0: [0], 1: [1], 2: [2], 3: [3], 9: [4], 12: [5]}
    v_at = {1: (0, 4), 4: (4, 8), 9: (8, 12), 13: (12, 16)}

    # --- greedy ACT/DVE balancer (running projected busy-ns per engine) ---
    bal = {'A': ACT_T0, 'D': DVE_T0}

    def pick(cols):
        ca, cd = 0.833 * cols + 185, 1.042 * cols + 125
        ta, td = bal['A'] + ca, bal['D'] + cd
        if ta <= td:
            bal['A'] = ta
            return 'A'
        bal['D'] = td
        return 'D'

    with tile.TileContext(nc) as tc:
        with tc.tile_pool(name="sb", bufs=1) as sb:
            cst_sb = sb.tile([128, 144], f16, tag="cst")
            nc.gpsimd.dma_start(cst_sb[:], cst_d.ap())  # SWDGE, own queue
            M_sb = cst_sb[:, 0:128]
            wv_sb = cst_sb[:, 128:144]

            xT = sb.tile([128, T], f16, tag="xT")
            for c0, cw in CH:
                nc.sync.dma_start(xT[:, c0:c0 + cw], xT_d.ap()[:, c0:c0 + cw])

            uS = sb.tile([128, T], f16, tag="uS")
            vS = sb.tile([128, NT, H + 1], f16, tag="vS")
            nc.gpsimd.memset(vS[:], 1.0)
            oS = sb.tile([128, NT, H + 1], f32, tag="oS")
            bias_sb = sb.tile([128, 1], f32, tag="bias")
            nc.vector.memset(bias_sb[:], MEAN_LN)

            warm = sb.tile([1, 2], f32, tag="warm")
            nc.vector.memset(warm[:, 0:1], 0.0)
            nc.scalar.activation(warm[:, 1:2], warm[:, 0:1], Exp)

            with (
                tc.tile_pool(name="psS", bufs=3, space="PSUM") as psS,
                tc.tile_pool(name="psP", bufs=1, space="PSUM") as psP,
                tc.tile_pool(name="pt", bufs=7) as ptp,
            ):
                po = psP.tile([128, NT * (H + 1) + 64], f32, tag="po",
                              name="po")
                pu = psP.tile([128, 512], f32, tag="pu", name="pu")

                def bal_copy(dst, src, cols):
                    if pick(cols) == 'A':
                        nc.scalar.copy(dst, src)
                        return 'A'
                    nc.vector.tensor_copy(dst, src)
                    return 'D'

                def emit_u(ci):
                    c0, cw = CH[ci]
                    nc.tensor.matmul(pu[:, 0:cw], M_sb, xT[:, c0:c0 + cw])
                    bal_copy(uS[:, c0:c0 + cw], pu[:, 0:cw], cw)

                def emit_v(j0, j1):
                    sc = po[:, NT * (H + 1):NT * (H + 1) + 16 * (j1 - j0)]
                    for u, j in enumerate(range(j0, j1)):
                        nc.tensor.matmul(
                            sc[:, 16 * u:16 * u + 16],
                            xT[:, 128 * j:128 * (j + 1)], wv_sb)
                    scv = sc.rearrange("p (u h) -> p u h", u=j1 - j0)
                    bal_copy(vS[:, j0:j1, 0:H], scv[:], 16 * (j1 - j0))

                pt_tiles = {}
                pv_left = {qt: qt + 1 for qt in range(NT)}
                pv_started = set()
                qb_gidx = {}
                for i, g in enumerate(groups):
                    qb_gidx.setdefault(g['qb'], []).append(i)
                qb_last_idx = {qb: gl[-1] for qb, gl in qb_gidx.items()}

                def emit_group(idx):
                    g = groups[idx]
                    st = psS.tile([128, GW], f32, tag="st")
                    a, b = g['a'], g['b']
                    for (j, off, wid, kind) in g['tiles']:
                        if kind == 'o':
                            mv = uS[:, 128 * b:128 * b + 128]
                        else:
                            mv = uS[:, 128 * a:128 * a + 256]
                        nc.tensor.matmul(st[:, off:off + wid],
                                         xT[:, 128 * j:128 * (j + 1)], mv)
                    pt = ptp.tile([128, GW], f16, tag="pt")
                    pt_tiles[idx] = pt
                    cols = g['cols']
                    if pick(cols) == 'A':
                        nc.scalar.activation(pt[:, 0:cols], st[:, 0:cols],
                                             Exp, scale=SCALE, bias=bias_sb[:])
                    else:
                        nc.vector.tensor_scalar(
                            pt[:, 0:cols].bitcast(i16), st[:, 0:cols],
                            MULT, MAGIC,
                            mybir.AluOpType.mult, mybir.AluOpType.add)
                    for (j, off, wid, kind) in g['tiles']:
                        if kind in ('e', 'o'):
                            nc.gpsimd.affine_select(
                                out=pt[:, off:off + 128],
                                in_=pt[:, off:off + 128],
                                compare_op=mybir.AluOpType.is_ge, fill=0.0,
                                base=0, pattern=[[1, 128]],
                                channel_multiplier=-1)

                def pv_mm(qt, pt_ap, j):
                    first = qt not in pv_started
                    if first:
                        pv_started.add(qt)
                    pv_left[qt] -= 1
                    nc.tensor.matmul(
                        po[:, (H + 1) * qt:(H + 1) * qt + H + 1],
                        pt_ap, vS[:, j, :],
                        start=first, stop=(pv_left[qt] == 0))

                def emit_pv_qb(qb):
                    a, b = 2 * qb, 2 * qb + 1
                    for qt in (a, b):
                        for gi in qb_gidx[qb]:
                            g = groups[gi]
                            pt = pt_tiles[gi]
                            for (j, off, wid, kind) in g['tiles']:
                                if kind == 'o':
                                    if qt == b:
                                        pv_mm(b, pt[:, off:off + 128], j)
                                elif qt == a:
                                    pv_mm(a, pt[:, off:off + 128], j)
                                else:
                                    pv_mm(b, pt[:, off + 128:off + 256], j)
                    for gi in qb_gidx[qb]:
                        pt_tiles.pop(gi)
                    if qb % 2 == 1:
                        q0 = 4 * (qb // 2)
                        eng = bal_copy(
                            oS[:, q0:q0 + 4, :],
                            po[:, (H + 1) * q0:(H + 1) * (q0 + 4)].rearrange(
                                "p (u h) -> p u h", u=4), 4 * (H + 1))
                        dst = out_d.ap().rearrange("(i p) h -> p i h", p=128)
                        dq = nc.scalar if eng == 'A' else nc.sync
                        dq.dma_start(dst[:, q0:q0 + 4, :], oS[:, q0:q0 + 4, :])

                pvq = []
                for idx in range(n):
                    for ci in u_at.get(idx, []):
                        emit_u(ci)
                    emit_group(idx)
                    if idx in v_at:
                        emit_v(*v_at[idx])
                    qb = groups[idx]['qb']
                    if idx == qb_last_idx[qb]:
                        pvq.append(qb)
                    while pvq and qb_last_idx[pvq[0]] <= idx - PD:
                        emit_pv_qb(pvq.pop(0))
                while pvq:
                    emit_pv_qb(pvq.pop(0))

    nc.finalize()
    return nc


def _get_nc(prec: str = "f16"):
    if "nc" not in _CACHE:
        _CACHE["nc"] = _build()
    return _CACHE["nc"]


def _host_inputs(Wq, Wk, Wv):
    Wq = np.asarray(Wq, dtype=np.float64)
    Wk = np.asarray(Wk, dtype=np.float64)
    Wv = np.asarray(Wv, dtype=np.float64)
    cst = np.zeros((128, 144), np.float16)
    cst[:, 0:128] = (Wq @ Wk.T).astype(np.float16)
    cst[:, 128:144] = Wv.astype(np.float16)
    return cst


def kernel(inpEmb, Wq, Wk, Wv):
    from concourse.bass_utils import run_bass_kernel_spmd

    nc = _get_nc()
    cst = _host_inputs(Wq, Wk, Wv)
    x = np.asarray(inpEmb, dtype=np.float32)
    in_maps = [
        {"xT": np.ascontiguousarray(x[b].T.astype(np.float16)), "cst": cst}
        for b in range(B)
    ]

    def run_and_check():
        br = run_bass_kernel_spmd(nc, in_maps, list(range(B)))
        out = np.empty((B, T, H), np.float32)
        for b in range(B):
            oD = br.results[b]["oD"]
            den = oD[:, H]
            if not (np.isfinite(oD).all() and (den > 0.0).all()):
                raise RuntimeError(f"core {b}: invalid kernel output")
            out[b] = oD[:, :H] / den[:, None]
        return out

    for attempt in range(3):
        try:
            return run_and_check()
        except Exception:
            if attempt == 2:
                raise


# revision 12
# speedup vs baseline: 1.0180x; 1.0180x over previous
"""Causal single-head attention (B=8, T=2048, D=128, H=16) on 8 Trainium2 cores.

Strategy (v3): data-parallel over batch (1 element per core). Per core:
  - Host precomputes M = Wq @ Wk^T [128,128]; the device computes
    u = M^T x (one matmul per x chunk), then score tiles
    ST[k, q] = xT_ktile^T @ u with K=128 contraction: no separate q/k
    projections, and x tiles serve directly as the ST stationary.
  - exp of each score group runs entirely on ONE of ACT/DVE, chosen by a
    greedy load balancer (whole-group: two engines writing one pt tile
    would serialize on a tile-granular WAW dependency). ACT groups use
    exact exp with a +ln(1.0407) bias so their systematic mean matches the
    DVE groups' piecewise-linear exp (i16 = round(score*scale*log2e*1024 +
    15360) bitcast f16); the common mean cancels in the softmax division.
    PSUM->SBUF copies (u/v/o) are balanced across ACT/DVE the same way.
  - Causal masking: odd-diagonal key tiles computed at half width; the two
    [128,128] triangular masks per query block applied in-place on the f16
    probability tiles by GPSIMD affine_select (idle engine).
  - PV uses pt as the matmul *stationary* and v-tiles [128, 17] (ones
    column -> softmax denominator) as 17-column moving operands. Output
    accumulates in one PSUM bank as 16 [128,17] regions; all matmuls of
    one region are emitted contiguously (PSUM allows only one open
    accumulation group per bank). Natural [T, 17] output layout, batched
    DMAs issued from the same engine queue as the preceding copy.
  - Host divides by the denominator column. rel l2 err ~3e-3.
"""

import os

import numpy as np

B, T, D, H = 8, 2048, 128, 16
NT = T // 128
SCALE = H ** -0.5
LOG2E = 1.4426950408889634
MULT = SCALE * LOG2E * 1024.0
MAGIC = 15360.0
MEAN_LN = 0.03987866060337333

GROUP = int(os.environ.get("ATT_GROUP", "4"))
PD = int(os.environ.get("ATT_PD", "2"))
ACT_T0 = float(os.environ.get("ATT_ACT_T0", "1583"))
DVE_T0 = float(os.environ.get("ATT_DVE_T0", "0"))

_CACHE = {}


def _groups():
    out = []
    for qb in range(8):
        a, b = 2 * qb, 2 * qb + 1
        tl = [(j, 256, 'f') for j in range(a)] + [(a, 256, 'e'), (b, 128, 'o')]
        ch = [tl[i:i + GROUP] for i in range(0, len(tl), GROUP)]
        ch.reverse()
        if qb == 7 and len(ch[-1]) >= 2:
            last = ch.pop()
            ch.append(last[1:])
            ch.append(last[:1])
        for tiles in ch:
            off = 0
            placed = []
            for (j, wid, kind) in tiles:
                placed.append((j, off, wid, kind))
                off += wid
            out.append(dict(qb=qb, a=a, b=b, tiles=placed, cols=off))
    return out


def _build():
    import concourse.mybir as mybir
    import concourse.tile as tile
    from concourse import bacc

    f32 = mybir.dt.float32
    f16 = mybir.dt.float16
    i16 = mybir.dt.int16
    Exp = mybir.ActivationFunctionType.Exp

    nc = bacc.Bacc()
    xT_d = nc.declare_dram_parameter("xT", [D, T], f16, isOutput=False)
    cst_d = nc.declare_dram_parameter("cst", [128, 144], f16, isOutput=False)
    out_d = nc.declare_dram_parameter("oD", [T, H + 1], f32, isOutput=True)

    groups = _groups()
    n = len(groups)
    GW = GROUP * 256

    # x / u chunk layout (finer early chunks fill the pipeline sooner)
    CH = [(0, 256), (256, 256), (512, 256), (768, 512), (1280, 512),
          (1792, 256)]
    u_at = {0: [0], 1: [1], 2: [2], 3: [3], 9: [4], 12: [5]}
    v_at = {1: (0, 4), 4: (4, 8), 9: (8, 12), 13: (12, 16)}

    # --- greedy ACT/DVE balancer (running projected busy-ns per engine) ---
    bal = {'A': ACT_T0, 'D': DVE_T0}

    def pick(cols):
        ca, cd = 0.833 * cols + 185, 1.042 * cols + 125
        ta, td = bal['A'] + ca, bal['D'] + cd
        if ta <= td:
            bal['A'] = ta
            return 'A'
        bal['D'] = td
        return 'D'

    with tile.TileContext(nc) as tc:
        with tc.tile_pool(name="sb", bufs=1) as sb:
            cst_sb = sb.tile([128, 144], f16, tag="cst")
            nc.gpsimd.dma_start(cst_sb[:], cst_d.ap())  # SWDGE, own queue
            M_sb = cst_sb[:, 0:128]
            wv_sb = cst_sb[:, 128:144]

            xT = sb.tile([128, T], f16, tag="xT")
            for c0, cw in CH:
                nc.sync.dma_start(xT[:, c0:c0 + cw], xT_d.ap()[:, c0:c0 + cw])

            uS = sb.tile([128, T], f16, tag="uS")
            vS = sb.tile([128, NT, H + 1], f16, tag="vS")
            nc.gpsimd.memset(vS[:], 1.0)
            oS = sb.tile([128, NT, H + 1], f32, tag="oS")
            bias_sb = sb.tile([128, 1], f32, tag="bias")
            nc.vector.memset(bias_sb[:], MEAN_LN)

            warm = sb.tile([1, 2], f32, tag="warm")
            nc.vector.memset(warm[:, 0:1], 0.0)
            nc.scalar.activation(warm[:, 1:2], warm[:, 0:1], Exp)

            with (
                tc.tile_pool(name="psS", bufs=3, space="PSUM") as psS,
                tc.tile_pool(name="psP", bufs=1, space="PSUM") as psP,
                tc.tile_pool(name="pt", bufs=7) as ptp,
            ):
                po = psP.tile([128, NT * (H + 1) + 64], f32, tag="po",
                              name="po")
                pu = psP.tile([128, 512], f32, tag="pu", name="pu")

                def bal_copy(dst, src, cols):
                    if pick(cols) == 'A':
                        nc.scalar.copy(dst, src)
                        return 'A'
                    nc.vector.tensor_copy(dst, src)
                    return 'D'

                def emit_u(ci):
                    c0, cw = CH[ci]
                    nc.tensor.matmul(pu[:, 0:cw], M_sb, xT[:, c0:c0 + cw])
                    bal_copy(uS[:, c0:c0 + cw], pu[:, 0:cw], cw)

                def emit_v(j0, j1):
                    sc = po[:, NT * (H + 1):NT * (H + 1) + 16 * (j1 - j0)]
                    for u, j in enumerate(range(j0, j1)):
                        nc.tensor.matmul(
                            sc[:, 16 * u:16 * u + 16],
                            xT[:, 128 * j:128 * (j + 1)], wv_sb)
                    scv = sc.rearrange("p (u h) -> p u h", u=j1 - j0)
                    bal_copy(vS[:, j0:j1, 0:H], scv[:], 16 * (j1 - j0))

                pt_tiles = {}
                pv_left = {qt: qt + 1 for qt in range(NT)}
                pv_started = set()
                qb_gidx = {}
                for i, g in enumerate(groups):
                    qb_gidx.setdefault(g['qb'], []).append(i)
                qb_last_idx = {qb: gl[-1] for qb, gl in qb_gidx.items()}

                def emit_group(idx):
                    g = groups[idx]
                    st = psS.tile([128, GW], f32, tag="st")
                    a, b = g['a'], g['b']
                    for (j, off, wid, kind) in g['tiles']:
                        if kind == 'o':
                            mv = uS[:, 128 * b:128 * b + 128]
                        else:
                            mv = uS[:, 128 * a:128 * a + 256]
                        nc.tensor.matmul(st[:, off:off + wid],
                                         xT[:, 128 * j:128 * (j + 1)], mv)
                    pt = ptp.tile([128, GW], f16, tag="pt")
                    pt_tiles[idx] = pt
                    cols = g['cols']
                    if pick(cols) == 'A':
                        nc.scalar.activation(pt[:, 0:cols], st[:, 0:cols],
                                             Exp, scale=SCALE, bias=bias_sb[:])
                    else:
                        nc.vector.tensor_scalar(
                            pt[:, 0:cols].bitcast(i16), st[:, 0:cols],
                            MULT, MAGIC,
                            mybir.AluOpType.mult, mybir.AluOpType.add)
                    for (j, off, wid, kind) in g['tiles']:
                        if kind in ('e', 'o'):
                            nc.gpsimd.affine_select(
                                out=pt[:, off:off + 128],
                                in_=pt[:, off:off + 128],
                                compare_op=mybir.AluOpType.is_ge, fill=0.0,
                                base=0, pattern=[[1, 128]],
                                channel_multiplier=-1)

                def pv_mm(qt, pt_ap, j):
                    first = qt not in pv_started
                    if first:
                        pv_started.add(qt)
                    pv_left[qt] -= 1
                    nc.tensor.matmul(
                        po[:, (H + 1) * qt:(H + 1) * qt + H + 1],
                        pt_ap, vS[:, j, :],
                        start=first, stop=(pv_left[qt] == 0))

                def emit_pv_qb(qb):
                    a, b = 2 * qb, 2 * qb + 1
                    for qt in (a, b):
                        for gi in qb_gidx[qb]:
                            g = groups[gi]
                            pt = pt_tiles[gi]
                            for (j, off, wid, kind) in g['tiles']:
                                if kind == 'o':
                                    if qt == b:
                                        pv_mm(b, pt[:, off:off + 128], j)
                                elif qt == a:
                                    pv_mm(a, pt[:, off:off + 128], j)
                                else:
                                    pv_mm(b, pt[:, off + 128:off + 256], j)
                    for gi in qb_gidx[qb]:
                        pt_tiles.pop(gi)
                    if qb % 2 == 1:
                        q0 = 4 * (qb // 2)
                        eng = bal_copy(
                            oS[:, q0:q0 + 4, :],
                            po[:, (H + 1) * q0:(H + 1) * (q0 + 4)].rearrange(
                                "p (u h) -> p u h", u=4), 4 * (H + 1))
                        dst = out_d.ap().rearrange("(i p) h -> p i h", p=128)
                        del eng  # DMA always from the idle SP queue: issuing
                        # from ACT/DVE would block that sequencer ~630ns
                        nc.sync.dma_start(dst[:, q0:q0 + 4, :],
                                          oS[:, q0:q0 + 4, :])

                pvq = []
                for idx in range(n):
                    for ci in u_at.get(idx, []):
                        emit_u(ci)
                    emit_group(idx)
                    if idx in v_at:
                        emit_v(*v_at[idx])
                    qb = groups[idx]['qb']
                    if idx == qb_last_idx[qb]:
                        pvq.append(qb)
                    while pvq and qb_last_idx[pvq[0]] <= idx - PD:
                        emit_pv_qb(pvq.pop(0))
                while pvq:
                    emit_pv_qb(pvq.pop(0))

    nc.finalize()
    return nc


def _get_nc(prec: str = "f16"):
    if "nc" not in _CACHE:
        _CACHE["nc"] = _build()
    return _CACHE["nc"]


def _host_inputs(Wq, Wk, Wv):
    Wq = np.asarray(Wq, dtype=np.float64)
    Wk = np.asarray(Wk, dtype=np.float64)
    Wv = np.asarray(Wv, dtype=np.float64)
    cst = np.zeros((128, 144), np.float16)
    cst[:, 0:128] = (Wq @ Wk.T).astype(np.float16)
    cst[:, 128:144] = Wv.astype(np.float16)
    return cst


def kernel(inpEmb, Wq, Wk, Wv):
    from concourse.bass_utils import run_bass_kernel_spmd

    nc = _get_nc()
    cst = _host_inputs(Wq, Wk, Wv)
    x = np.asarray(inpEmb, dtype=np.float32)
    in_maps = [
        {"xT": np.ascontiguousarray(x[b].T.astype(np.float16)), "cst": cst}
        for b in range(B)
    ]

    def run_and_check():
        br = run_bass_kernel_spmd(nc, in_maps, list(range(B)))
        out = np.empty((B, T, H), np.float32)
        for b in range(B):
            oD = br.results[b]["oD"]
            den = oD[:, H]
            if not (np.isfinite(oD).all() and (den > 0.0).all()):
                raise RuntimeError(f"core {b}: invalid kernel output")
            out[b] = oD[:, :H] / den[:, None]
        return out

    for attempt in range(3):
        try:
            return run_and_check()
        except Exception:
            if attempt == 2:
                raise
